# revision 6
# baseline (speedup 1.0000x reference)
"""AdaptiveConv2DMod Trainium2 kernel — 1-D Winograd F(2,3) along y.

Data-parallel over batch b=8 across 8 NeuronCores.

Per core (o-chunk q, i-chunk c, 128 channels each):
  sel = softmax(embed @ adapt_w.T + adapt_b)
  wsc[n] = bf16(sel_n * weights[n])   (ACT copy with per-partition scale --
           fp32 scalar ops are fast on ACT; bf16 PTR-scalar ops on the DVE
           hit a slow microcode path, so sel is folded into the cast)
  v   = (wsc0+wsc1) + (wsc2+wsc3)     (DVE pure-TT adds, bf16 fast path)
  transposes -> wct[c] = [i, tap, o] bf16, modulated by mscale[i] via the
  ACT evacuation scale (partition dim is i).
  inv_norm[o] = rsqrt(clip(sum msq[i] * v^2, eps))  (DVE, pre-transpose)

Winograd along y, G'' = [[1,0,0],[1,1,1],[1,-1,1],[0,0,1]] (adds only; the
0.5 compensation rides the z-evacuation scale of the u=1,2 banks):
  weight:  u0/u3 alias wct rows ky=0/ky=2; u1 = k0+k1+k2, u2 = k0-k1+k2
  input:   V[c][u, ty, x] built straight from the dense fst staging tile;
           border tiles get dedicated single-row ops, x-borders memset.
  GEMM:    M[u][o, ty, x] += U[u,kx][i,o].T @ V[u][i, ty, x+kx]
           per (q, ty-block of 8): 4 u-banks, 48 matmuls, PSUM tags
           m{u}_{tb%2} (8 banks); weight transposes for the next q borrow
           the just-evacuated m{g}_{tb%2} slots.
  output:  z[u] = s_u * inv_norm * M[u] (ACT evac, bf16, s_u = 0.5 for
           u=1,2), y_even = (z1+z2) + z0, y_odd = (z1-z2) - z3 — pure TTs
           into a contiguous [128, 2, 8, 64] bf16 block; the gpsimd output
           DMA casts to fp32 and interleaves the even/odd rows via its
           destination access pattern.

Emission is software-pipelined: the weight block for (q+1, c=tb) is
emitted right after ty-block tb of q, so combines/transposes overlap the
GEMM sweep and the PE never waits at q boundaries.

Cuts PE matmul work 1.5x vs direct conv (768 vs 1152 N=512 matmuls).

Queue discipline: small params FIRST on the sync ring, then bulk weights
(per-bank split for (q0,c0) to cut first-combine latency); fmap on gpsimd
(the only casting DGE) into DENSE staging tiles.  sel is computed
redundantly on all 128 partitions (adapt_w broadcast by a DRE DMA) so no
cross-partition broadcast op is needed; mscale^2 broadcasts via a DRAM
round-trip on the gpsimd ring.  Output blocks are written row-interleaved
by strided fp32 DVE TTs and leave as contiguous DMAs on the scalar ring.
"""

import sys

if "/opt/trn_rl_repo" not in sys.path:
    sys.path.insert(0, "/opt/trn_rl_repo")

import numpy as np

import concourse.bass as bass
import concourse.tile as tile
from concourse import bacc, mybir
from concourse.bass_utils import run_bass_kernel_spmd
from concourse.masks import make_identity

F32 = mybir.dt.float32
BF16 = mybir.dt.bfloat16

O, I, H, W, KS, NB = 512, 512, 64, 64, 3, 4
OC = O // 128
IC = I // 128
TY = H // 2     # winograd y-tiles (32)
PXV = W + 4     # V row pitch (68: zero cols 0:2, interior 2:66, pad 66:68)
TB = 8          # ty rows per psum block
NTB = TY // TB  # ty-blocks per q (4), each = 16 output rows
EPS = 1e-8

_CACHED = {}


def _build():
    nc = bacc.Bacc("TRN2", target_bir_lowering=False, debug=False, num_devices=8)

    fmap = nc.dram_tensor("fmap", [I, H, W], F32, kind="ExternalInput").ap()
    embed = nc.dram_tensor("embed", [512], F32, kind="ExternalInput").ap()
    weights = nc.dram_tensor("weights", [NB, O, I, KS, KS], F32, kind="ExternalInput").ap()
    mod_w = nc.dram_tensor("mod_w", [512, 512], F32, kind="ExternalInput").ap()
    mod_b = nc.dram_tensor("mod_b", [512], F32, kind="ExternalInput").ap()
    adapt_w = nc.dram_tensor("adapt_w", [NB, 512], F32, kind="ExternalInput").ap()
    adapt_b = nc.dram_tensor("adapt_b", [NB], F32, kind="ExternalInput").ap()
    out = nc.dram_tensor("out", [O, H, W], F32, kind="ExternalOutput").ap()
    msq_dram = nc.dram_tensor("msq_scratch", [I], F32, kind="Internal").ap()
    sel_dram = nc.dram_tensor("sel_scratch", [NB], F32, kind="Internal").ap()

    with tile.TileContext(nc) as tc:
        _emit(nc, tc, fmap, embed, weights, mod_w, mod_b, adapt_w, adapt_b,
              out, msq_dram, sel_dram)

    nc.compile()
    return nc


def _emit(nc, tc, fmap, embed, weights, mod_w, mod_b, adapt_w, adapt_b, out,
          msq_dram, sel_dram):
    import contextlib

    ctx = contextlib.ExitStack()
    with ctx:
        const = ctx.enter_context(tc.tile_pool(name="const", bufs=1))
        small = ctx.enter_context(tc.tile_pool(name="small", bufs=2))
        mw_p = ctx.enter_context(tc.tile_pool(name="mw", bufs=4))
        fst_p = ctx.enter_context(tc.tile_pool(name="fst", bufs=1))
        wbank_p = ctx.enter_context(tc.tile_pool(name="wbank", bufs=2))
        wsc_p = ctx.enter_context(tc.tile_pool(name="wsc", bufs=1))
        v_p = ctx.enter_context(tc.tile_pool(name="v", bufs=2))
        vm_p = ctx.enter_context(tc.tile_pool(name="vm", bufs=1))
        wct_p = ctx.enter_context(tc.tile_pool(name="wct", bufs=2))
        uw_p = ctx.enter_context(tc.tile_pool(name="uw", bufs=2))
        z_p = ctx.enter_context(tc.tile_pool(name="z", bufs=2))
        ob_p = ctx.enter_context(tc.tile_pool(name="ob", bufs=2))
        ps_p = ctx.enter_context(tc.tile_pool(name="ps", bufs=1, space="PSUM"))

        # ---------------- small param DMAs FIRST on the sync ring ---------
        embed_b = const.tile([128, 512], F32, tag="embed_b")
        nc.scalar.dma_start(
            out=embed_b,
            in_=bass.AP(tensor=embed.tensor, offset=embed.offset,
                        ap=[[0, 128], [1, 512]]),
        )
        aw_b = const.tile([128, NB, 512], F32, tag="aw_b")
        nc.sync.dma_start(
            out=aw_b,
            in_=bass.AP(tensor=adapt_w.tensor, offset=adapt_w.offset,
                        ap=[[0, 128], [1, NB * 512]]),
        )
        ab_b = const.tile([128, NB], F32, tag="ab_b")
        nc.sync.dma_start(
            out=ab_b,
            in_=bass.AP(tensor=adapt_b.tensor, offset=adapt_b.offset,
                        ap=[[0, 128], [1, NB]]),
        )
        mw_l = []
        for c in range(4):
            mw = mw_p.tile([128, 512], F32, tag="mw", name=f"mw{c}")
            nc.sync.dma_start(out=mw, in_=mod_w[c * 128:(c + 1) * 128, :])
            mw_l.append(mw)
        modb_t = const.tile([128, 4], F32, tag="modb_t")
        nc.sync.dma_start(
            out=modb_t,
            in_=bass.AP(tensor=mod_b.tensor, offset=mod_b.offset,
                        ap=[[1, 128], [128, 4]]),
        )


        # ---------------- fmap staging (dense bf16, gpsimd caster) --------
        fst_l = [fst_p.tile([128, H, W], BF16, tag="fst", name=f"fst{c}")
                 for c in range(IC)]
        nc.gpsimd.dma_start(out=fst_l[0], in_=fmap[0:128, :, :])
        ident = const.tile([128, 128], BF16, tag="ident")
        make_identity(nc, ident)

        vt_l = [const.tile([128, 4, TY, PXV], BF16, tag=f"vt{c}",
                           name=f"vt{c}") for c in range(IC)]
        for c in range(IC):
            vt = vt_l[c][:, :, :, :]
            for off, n in ((0, 2), (W + 2, 2)):
                nc.vector.memset(
                    bass.AP(tensor=vt.tensor, offset=vt.offset + off,
                            ap=[vt.ap[0], [PXV, 4 * TY], [1, n]]), 0.0)

        def emit_v_transform(c):
            # V rows from dense fst (img rows r = 2ty + delta - 1):
            #   V0[ty] = d(2ty-1) - d(2ty+1)   main ty=1..31, V0[0] = -d(1)
            #   V1[ty] = d(2ty)   + d(2ty+1)   all ty
            #   V2[ty] = d(2ty+1) - d(2ty)     all ty
            #   V3[ty] = d(2ty)   - d(2ty+2)   main ty=0..30, V3[31] = d(62)
            fs = fst_l[c][:, :, :]
            vt = vt_l[c][:, :, :, :]

            def rows(r0, n):
                return bass.AP(tensor=fs.tensor, offset=fs.offset + r0 * W,
                               ap=[fs.ap[0], [2 * W, n], [1, W]])

            def vout(u, ty0, n):
                return bass.AP(tensor=vt.tensor,
                               offset=vt.offset + u * TY * PXV + ty0 * PXV + 2,
                               ap=[vt.ap[0], [PXV, n], [1, W]])

            TT = nc.vector.tensor_tensor
            sub, add = mybir.AluOpType.subtract, mybir.AluOpType.add
            TT(out=vout(0, 1, 31), in0=rows(1, 31), in1=rows(3, 31), op=sub)
            nc.vector.tensor_scalar_mul(out=vout(0, 0, 1), in0=rows(1, 1),
                                        scalar1=-1.0)
            TT(out=vout(1, 0, 32), in0=rows(0, 32), in1=rows(1, 32), op=add)
            TT(out=vout(2, 0, 32), in0=rows(1, 32), in1=rows(0, 32), op=sub)
            TT(out=vout(3, 0, 31), in0=rows(0, 31), in1=rows(2, 31), op=sub)
            nc.vector.tensor_copy(out=vout(3, 31, 1), in_=rows(62, 1))

        # -------- sel = softmax(embed @ adapt_w.T + adapt_b), computed
        # redundantly on every partition (adapt_w broadcast by the DMA) so
        # no cross-partition broadcast is ever needed -------------------
        lgt = const.tile([128, NB], F32, tag="lgt")
        for n in range(NB):
            junk4 = small.tile([128, 512], F32, tag="junk", bufs=1,
                               name=f"junk4_{n}")
            nc.vector.scalar_tensor_tensor(
                out=junk4, in0=aw_b[:, n, :], scalar=1.0, in1=embed_b,
                op0=mybir.AluOpType.bypass, op1=mybir.AluOpType.mult,
                accum_out=lgt[:, n:n + 1],
            )
        nc.vector.tensor_tensor(out=lgt, in0=lgt, in1=ab_b,
                                op=mybir.AluOpType.add)
        ex = const.tile([128, NB], F32, tag="ex")
        nc.scalar.activation(out=ex, in_=lgt,
                             func=mybir.ActivationFunctionType.Exp,
                             bias=0.0, scale=1.0)
        sm = const.tile([128, 1], F32, tag="sm")
        nc.vector.tensor_reduce(out=sm, in_=ex, axis=mybir.AxisListType.X,
                                op=mybir.AluOpType.add)
        rs = const.tile([128, 1], F32, tag="rs")
        nc.vector.reciprocal(out=rs, in_=sm)
        sel_b = const.tile([128, NB], F32, tag="sel_b")
        nc.vector.tensor_scalar_mul(out=sel_b, in0=ex, scalar1=rs)

        # mod/mscale tiles (DMAs + compute emitted after the first weight
        # block so the sync ring streams wbig-c0 first and the DVE runs the
        # sel chain unobstructed)
        mod_t = const.tile([128, 4], F32, tag="mod_t")
        msc = const.tile([128, 4], F32, tag="msc")
        msq_t = const.tile([128, 4], F32, tag="msq_t")
        msq_f = [const.tile([128, 128], F32, tag=f"m9f{c}", name=f"m9f{c}")
                 for c in range(IC)]
        msq9 = [None] * IC

        def emit_mod_chain():
            for c in range(4):
                junk = small.tile([128, 512], F32, tag="junk", bufs=1,
                                  name=f"junkm{c}")
                nc.vector.scalar_tensor_tensor(
                    out=junk, in0=mw_l[c], scalar=1.0, in1=embed_b,
                    op0=mybir.AluOpType.bypass, op1=mybir.AluOpType.mult,
                    accum_out=mod_t[:, c:c + 1],
                )
            nc.vector.scalar_tensor_tensor(
                out=msc, in0=mod_t, scalar=1.0, in1=modb_t,
                op0=mybir.AluOpType.add, op1=mybir.AluOpType.add,
            )
            nc.vector.tensor_tensor(out=msq_t, in0=msc, in1=msc,
                                    op=mybir.AluOpType.mult)
            nc.gpsimd.dma_start(
                out=bass.AP(tensor=msq_dram.tensor, offset=msq_dram.offset,
                            ap=[[1, 128], [128, 4]]),
                in_=msq_t,
            )
            for c in range(IC):
                nc.gpsimd.dma_start(
                    out=msq_f[c],
                    in_=bass.AP(tensor=msq_dram.tensor,
                                offset=msq_dram.offset + c * 128,
                                ap=[[0, 128], [1, 128]]),
                )

        inv_norm = const.tile([128, OC], F32, tag="inv_norm")
        inv_half = const.tile([128, OC], F32, tag="inv_half")

        wct_d, uw_d, ntp_d = {}, {}, {}

        def weights_block(q, c, emit_norm=True):
            if c == 0:
                wct_d[q] = [wct_p.tile([128, KS * KS, 128], BF16,
                                       tag=f"wct{cc}", name=f"wct{cc}_{q}")
                            for cc in range(IC)]
                uw_d[q] = [uw_p.tile([128, 6, 128], BF16, tag=f"uw{cc}",
                                     name=f"uw{cc}_{q}") for cc in range(IC)]
                ntp_d[q] = small.tile([128, IC], F32, tag="ntp",
                                      name=f"ntp{q}")
            wct_q, uw_q, ntp = wct_d[q], uw_d[q], ntp_d[q]
            wbig = wbank_p.tile([128, NB, 128 * KS * KS], F32, tag="wbig",
                                name=f"wbig{q}_{c}")
            nsplit = NB if (q == 0 and c == 0) else 1
            bper = NB // nsplit
            for sp in range(nsplit):
                nc.sync.dma_start(
                    out=wbig[:, sp * bper:(sp + 1) * bper, :],
                    in_=bass.AP(
                        tensor=weights.tensor,
                        offset=(weights.offset + q * 128 * I * KS * KS
                                + sp * bper * O * I * KS * KS
                                + c * 128 * KS * KS),
                        ap=[[I * KS * KS, 128], [O * I * KS * KS, bper],
                            [1, 128 * KS * KS]],
                    ),
                )
            # fold sel into the bf16 cast (ACT, per-partition fp32 scale)
            wsc = wsc_p.tile([128, NB, 128 * KS * KS], BF16, tag="wsc",
                             name=f"wsc{q}_{c}")
            for n in range(NB):
                nc.scalar.activation(
                    out=wsc[:, n, :], in_=wbig[:, n, :],
                    func=mybir.ActivationFunctionType.Copy,
                    scale=sel_b[:, n:n + 1],
                )
            # combine: pure bf16 TT adds (fast path); v dense [o, i, tap]
            t0 = small.tile([128, 128 * KS * KS], BF16, tag="t0", bufs=1)
            nc.vector.tensor_tensor(out=t0, in0=wsc[:, 0, :],
                                    in1=wsc[:, 1, :], op=mybir.AluOpType.add)
            t1 = small.tile([128, 128 * KS * KS], BF16, tag="t1", bufs=1)
            nc.vector.tensor_tensor(out=t1, in0=wsc[:, 2, :],
                                    in1=wsc[:, 3, :], op=mybir.AluOpType.add)
            v = v_p.tile([128, 128, KS * KS], BF16, tag="v",
                         name=f"v{q}_{c}")
            nc.vector.tensor_tensor(
                out=v, in0=t0.rearrange("p (i j) -> p i j", j=KS * KS),
                in1=t1.rearrange("p (i j) -> p i j", j=KS * KS),
                op=mybir.AluOpType.add)
            # taps [o,i]->[i,o], 3 groups of 3 into just-freed PSUM slots
            va = v[:, :, :]
            pts = []
            for g in range(3):
                pt = ps_p.tile([128, 3, 128], BF16, tag=f"m{g}_{c % 2}",
                               name=f"pt{g}_{q}_{c}")
                for j in range(3 * g, 3 * g + 3):
                    nc.tensor.transpose(
                        pt[:, j - 3 * g, :],
                        bass.AP(tensor=va.tensor, offset=va.offset + j,
                                ap=[va.ap[0], [KS * KS, 128]]),
                        ident)
                pts.append(pt)

            def tail():
                # ACT evacuation applies mscale (i on partitions)
                for g, pt in enumerate(pts):
                    nc.scalar.activation(
                        out=wct_q[c][:, 3 * g:3 * g + 3, :], in_=pt,
                        func=mybir.ActivationFunctionType.Copy,
                        scale=msc[:, c:c + 1],
                    )
                if q == 0:
                    emit_v_transform(c)
                # winograd weight transform (pure TT adds)
                st = small.tile([128, KS, 128], BF16, tag="st", bufs=1)
                nc.vector.tensor_tensor(out=st, in0=wct_q[c][:, 0:3, :],
                                        in1=wct_q[c][:, 6:9, :],
                                        op=mybir.AluOpType.add)
                nc.vector.tensor_tensor(out=uw_q[c][:, 0:3, :], in0=st,
                                        in1=wct_q[c][:, 3:6, :],
                                        op=mybir.AluOpType.add)
                nc.vector.tensor_tensor(out=uw_q[c][:, 3:6, :], in0=st,
                                        in1=wct_q[c][:, 3:6, :],
                                        op=mybir.AluOpType.subtract)
                weights_norm(q, c, v)

            if emit_norm:
                tail()
            return wbig, tail

        def weights_norm(q, c, v):
            # demod norm partial (off the critical path): sum v^2 * msq[i]
            ntp = ntp_d[q]
            if q == 0:
                msq9[c] = const.tile([128, 128], BF16, tag=f"m9b{c}",
                                     name=f"m9b{c}")
                nc.vector.tensor_copy(out=msq9[c], in_=msq_f[c])
            m9ap = msq9[c][:, :]
            m9b = bass.AP(tensor=m9ap.tensor, offset=m9ap.offset,
                          ap=[m9ap.ap[0], [1, 128], [0, KS * KS]])
            vm = vm_p.tile([128, 128, KS * KS], BF16, tag="vm")
            nc.vector.tensor_tensor(out=vm, in0=v, in1=m9b,
                                    op=mybir.AluOpType.mult)
            nc.vector.scalar_tensor_tensor(
                out=vm, in0=vm, scalar=1.0, in1=v,
                op0=mybir.AluOpType.bypass, op1=mybir.AluOpType.mult,
                accum_out=ntp[:, c:c + 1],
            )

        emit_mod_chain()
        wbig00, _ = weights_block(0, 0)
        # hold fst c1-3 DMAs until the first weight chunk has landed, so
        # the critical-path prologue traffic is not starved for bandwidth
        dummy = const.tile([1, 1], F32, tag="dummy")
        nc.gpsimd.tensor_copy(out=dummy, in_=wbig00[0:1, NB - 1, 0:1])
        for c in range(1, IC):
            nc.gpsimd.dma_start(out=fst_l[c],
                                in_=fmap[c * 128:(c + 1) * 128, :, :])
        for c in range(1, IC):
            weights_block(0, c)

        for q in range(OC):
            wct_q, uw_q, ntp = wct_d[q], uw_d[q], ntp_d[q]
            # inv_norm[q] = rsqrt(clip(sum_c ntp, EPS)); inv_half = half
            nt1 = small.tile([128, 1], F32, tag="nt1")
            nc.vector.tensor_reduce(out=nt1, in_=ntp, axis=mybir.AxisListType.X,
                                    op=mybir.AluOpType.add)
            nc.vector.tensor_scalar_max(out=nt1, in0=nt1, scalar1=EPS)
            nc.scalar.sqrt(out=nt1, in_=nt1)
            nc.vector.reciprocal(out=inv_norm[:, q:q + 1], in_=nt1)
            nc.vector.tensor_scalar_mul(out=inv_half[:, q:q + 1],
                                        in0=inv_norm[:, q:q + 1], scalar1=0.5)

            def tap(c, u, kx):
                if u == 0:
                    return wct_q[c][:, kx, :]
                if u == 3:
                    return wct_q[c][:, 6 + kx, :]
                return uw_q[c][:, (u - 1) * 3 + kx, :]

            def gemm_mm(mt, tb, c, u, kx):
                nc.tensor.matmul(
                    mt[u],
                    tap(c, u, kx),
                    vt_l[c][:, u, tb * TB:(tb + 1) * TB, kx + 1:kx + 1 + W],
                    start=(c == 0 and kx == 0),
                    stop=(c == IC - 1 and kx == KS - 1),
                    skip_group_check=True,
                )

            def out_block(tb, mt):
                # output transform: z = s_u * inv_norm * M (ACT), pure TTs
                z = z_p.tile([128, 4, 512], BF16, tag="z")
                for u in range(4):
                    sc = inv_half if u in (1, 2) else inv_norm
                    nc.scalar.activation(
                        out=z[:, u, :], in_=mt[u],
                        func=mybir.ActivationFunctionType.Copy,
                        scale=sc[:, q:q + 1],
                    )
                s = z_p.tile([128, 512], BF16, tag="s", bufs=1)
                d = z_p.tile([128, 512], BF16, tag="d", bufs=1)
                nc.vector.tensor_tensor(out=s, in0=z[:, 1, :], in1=z[:, 2, :],
                                        op=mybir.AluOpType.add)
                nc.vector.tensor_tensor(out=d, in0=z[:, 1, :], in1=z[:, 2, :],
                                        op=mybir.AluOpType.subtract)
                # DVE writes the row interleave directly (strided fp32
                # stays in 1x mode); one contiguous fp32 DMA per block
                ob = ob_p.tile([128, 2 * TB, W], F32, tag="ob")
                oba = ob[:, :, :]
                y_even = bass.AP(tensor=oba.tensor, offset=oba.offset,
                                 ap=[oba.ap[0], [2 * W, TB], [1, W]])
                y_odd = bass.AP(tensor=oba.tensor, offset=oba.offset + W,
                                ap=[oba.ap[0], [2 * W, TB], [1, W]])
                nc.vector.tensor_tensor(out=y_even, in0=s, in1=z[:, 0, :],
                                        op=mybir.AluOpType.add)
                nc.vector.tensor_tensor(out=y_odd, in0=d, in1=z[:, 3, :],
                                        op=mybir.AluOpType.subtract)
                nc.scalar.dma_start(
                    out=out[q * 128:(q + 1) * 128,
                            tb * 2 * TB:(tb + 1) * 2 * TB, :],
                    in_=ob,
                )

            def mk_mt(tb):
                return [ps_p.tile([128, 512], F32, tag=f"m{u}_{tb % 2}",
                                  name=f"m{u}_{q}_{tb}") for u in range(4)]

            # -------- GEMM sweep: 4 u-banks per ty-block, c/kx contracted.
            # q0 runs c-OUTER over tb-pairs so matmuls start as soon as the
            # first weight chunk is ready instead of waiting for all four.
            if q == 0:
                for pair in range(NTB // 2):
                    tbs = (2 * pair, 2 * pair + 1)
                    mts = {tb: mk_mt(tb) for tb in tbs}
                    for c in range(IC):
                        for tb in tbs:
                            for u in range(4):
                                for kx in range(KS):
                                    gemm_mm(mts[tb], tb, c, u, kx)
                    for tb in tbs:
                        out_block(tb, mts[tb])
                        weights_block(1, tb)
            else:
                for tb in range(NTB):
                    mt = mk_mt(tb)
                    for c in range(IC):
                        for u in range(4):
                            for kx in range(KS):
                                gemm_mm(mt, tb, c, u, kx)
                    out_block(tb, mt)
                    if q + 1 < OC:
                        weights_block(q + 1, tb)


def _get_nc():
    if "nc" not in _CACHED:
        _CACHED["nc"] = _build()
    return _CACHED["nc"]


def _run(inputs, trace=False):
    nc = _get_nc()
    fmap = np.ascontiguousarray(inputs["fmap"], dtype=np.float32)
    embed = np.ascontiguousarray(inputs["embed"], dtype=np.float32)
    shared = {
        "weights": np.ascontiguousarray(inputs["weights"], dtype=np.float32),
        "mod_w": np.ascontiguousarray(inputs["mod_w"], dtype=np.float32),
        "mod_b": np.ascontiguousarray(inputs["mod_b"], dtype=np.float32),
        "adapt_w": np.ascontiguousarray(inputs["adapt_w"], dtype=np.float32),
        "adapt_b": np.ascontiguousarray(inputs["adapt_b"], dtype=np.float32),
    }
    b = fmap.shape[0]
    in_maps = [
        {"fmap": np.ascontiguousarray(fmap[c]),
         "embed": np.ascontiguousarray(embed[c]), **shared}
        for c in range(b)
    ]
    res = run_bass_kernel_spmd(nc, in_maps, core_ids=list(range(b)),
                               trace=trace)
    _CACHED["last_res"] = res
    outs = np.stack([res.results[c]["out"] for c in range(b)], axis=0)
    return outs.astype(np.float32), res.exec_time_ns


def kernel(**inputs):
    out, _ = _run(inputs, trace=False)
    return out


def kernel_traced(**inputs):
    return _run(inputs, trace=True)


# revision 7
# speedup vs baseline: 1.0158x; 1.0158x over previous
"""AdaptiveConv2DMod Trainium2 kernel — 1-D Winograd F(2,3) along y.

Data-parallel over batch b=8 across 8 NeuronCores.

Per core (o-chunk q, i-chunk c, 128 channels each):
  sel = softmax(embed @ adapt_w.T + adapt_b)
  wsc[n] = bf16(sel_n * weights[n])   (ACT copy with per-partition scale --
           fp32 scalar ops are fast on ACT; bf16 PTR-scalar ops on the DVE
           hit a slow microcode path, so sel is folded into the cast)
  v   = (wsc0+wsc1) + (wsc2+wsc3)     (DVE pure-TT adds, bf16 fast path)
  transposes -> wct[c] = [i, tap, o] bf16, modulated by mscale[i] via the
  ACT evacuation scale (partition dim is i).
  inv_norm[o] = rsqrt(clip(sum msq[i] * v^2, eps))  (DVE, pre-transpose)

Winograd along y, G'' = [[1,0,0],[1,1,1],[1,-1,1],[0,0,1]] (adds only; the
0.5 compensation rides the z-evacuation scale of the u=1,2 banks):
  weight:  u0/u3 alias wct rows ky=0/ky=2; u1 = k0+k1+k2, u2 = k0-k1+k2
  input:   V[c][u, ty, x] built straight from the dense fst staging tile;
           border tiles get dedicated single-row ops, x-borders memset.
  GEMM:    M[u][o, ty, x] += U[u,kx][i,o].T @ V[u][i, ty, x+kx]
           per (q, ty-block of 8): 4 u-banks, 48 matmuls, PSUM tags
           m{u}_{tb%2} (8 banks); weight transposes for the next q borrow
           the just-evacuated m{g}_{tb%2} slots.
  output:  z[u] = s_u * inv_norm * M[u] (ACT evac, bf16, s_u = 0.5 for
           u=1,2), y_even = (z1+z2) + z0, y_odd = (z1-z2) - z3 — pure TTs
           into a contiguous [128, 2, 8, 64] bf16 block; the gpsimd output
           DMA casts to fp32 and interleaves the even/odd rows via its
           destination access pattern.

Emission is software-pipelined: the weight block for (q+1, c=tb) is
emitted right after ty-block tb of q, so combines/transposes overlap the
GEMM sweep and the PE never waits at q boundaries.

Cuts PE matmul work 1.5x vs direct conv (768 vs 1152 N=512 matmuls).

Queue discipline: small params FIRST on the sync ring, then bulk weights
(per-bank split for (q0,c0) to cut first-combine latency); fmap on gpsimd
(the only casting DGE) into DENSE staging tiles.  sel is computed
redundantly on all 128 partitions (adapt_w broadcast by a DRE DMA) so no
cross-partition broadcast op is needed; mscale^2 broadcasts via a DRAM
round-trip on the gpsimd ring.  Output blocks are written row-interleaved
by strided fp32 DVE TTs and leave as contiguous DMAs on the scalar ring.
"""

import sys

if "/opt/trn_rl_repo" not in sys.path:
    sys.path.insert(0, "/opt/trn_rl_repo")

import numpy as np

import concourse.bass as bass
import concourse.tile as tile
from concourse import bacc, mybir
from concourse.bass_utils import run_bass_kernel_spmd
from concourse.masks import make_identity

F32 = mybir.dt.float32
BF16 = mybir.dt.bfloat16

O, I, H, W, KS, NB = 512, 512, 64, 64, 3, 4
OC = O // 128
IC = I // 128
TY = H // 2     # winograd y-tiles (32)
PXV = W + 4     # V row pitch (68: zero cols 0:2, interior 2:66, pad 66:68)
TB = 8          # ty rows per psum block
NTB = TY // TB  # ty-blocks per q (4), each = 16 output rows
EPS = 1e-8

_CACHED = {}


def _build():
    nc = bacc.Bacc("TRN2", target_bir_lowering=False, debug=False, num_devices=8)

    fmap = nc.dram_tensor("fmap", [I, H, W], F32, kind="ExternalInput").ap()
    embed = nc.dram_tensor("embed", [512], F32, kind="ExternalInput").ap()
    weights = nc.dram_tensor("weights", [NB, O, I, KS, KS], F32, kind="ExternalInput").ap()
    mod_w = nc.dram_tensor("mod_w", [512, 512], F32, kind="ExternalInput").ap()
    mod_b = nc.dram_tensor("mod_b", [512], F32, kind="ExternalInput").ap()
    adapt_w = nc.dram_tensor("adapt_w", [NB, 512], F32, kind="ExternalInput").ap()
    adapt_b = nc.dram_tensor("adapt_b", [NB], F32, kind="ExternalInput").ap()
    out = nc.dram_tensor("out", [O, H, W], F32, kind="ExternalOutput").ap()
    msq_dram = nc.dram_tensor("msq_scratch", [I], F32, kind="Internal").ap()
    sel_dram = nc.dram_tensor("sel_scratch", [NB], F32, kind="Internal").ap()

    with tile.TileContext(nc) as tc:
        _emit(nc, tc, fmap, embed, weights, mod_w, mod_b, adapt_w, adapt_b,
              out, msq_dram, sel_dram)

    nc.compile()
    return nc


def _emit(nc, tc, fmap, embed, weights, mod_w, mod_b, adapt_w, adapt_b, out,
          msq_dram, sel_dram):
    import contextlib

    ctx = contextlib.ExitStack()
    with ctx:
        const = ctx.enter_context(tc.tile_pool(name="const", bufs=1))
        small = ctx.enter_context(tc.tile_pool(name="small", bufs=2))
        mw_p = ctx.enter_context(tc.tile_pool(name="mw", bufs=4))
        fst_p = ctx.enter_context(tc.tile_pool(name="fst", bufs=1))
        wbank_p = ctx.enter_context(tc.tile_pool(name="wbank", bufs=2))
        wsc_p = ctx.enter_context(tc.tile_pool(name="wsc", bufs=1))
        v_p = ctx.enter_context(tc.tile_pool(name="v", bufs=2))
        vm_p = ctx.enter_context(tc.tile_pool(name="vm", bufs=1))
        wct_p = ctx.enter_context(tc.tile_pool(name="wct", bufs=2))
        uw_p = ctx.enter_context(tc.tile_pool(name="uw", bufs=2))
        z_p = ctx.enter_context(tc.tile_pool(name="z", bufs=2))
        ob_p = ctx.enter_context(tc.tile_pool(name="ob", bufs=2))
        ps_p = ctx.enter_context(tc.tile_pool(name="ps", bufs=1, space="PSUM"))

        # ---------------- small param DMAs FIRST on the sync ring ---------
        embed_b = const.tile([128, 512], F32, tag="embed_b")
        nc.scalar.dma_start(
            out=embed_b,
            in_=bass.AP(tensor=embed.tensor, offset=embed.offset,
                        ap=[[0, 128], [1, 512]]),
        )
        aw_b = const.tile([128, NB, 512], F32, tag="aw_b")
        nc.sync.dma_start(
            out=aw_b,
            in_=bass.AP(tensor=adapt_w.tensor, offset=adapt_w.offset,
                        ap=[[0, 128], [1, NB * 512]]),
        )
        ab_b = const.tile([128, NB], F32, tag="ab_b")
        nc.sync.dma_start(
            out=ab_b,
            in_=bass.AP(tensor=adapt_b.tensor, offset=adapt_b.offset,
                        ap=[[0, 128], [1, NB]]),
        )
        mw_l = []
        for c in range(4):
            mw = mw_p.tile([128, 512], F32, tag="mw", name=f"mw{c}")
            nc.sync.dma_start(out=mw, in_=mod_w[c * 128:(c + 1) * 128, :])
            mw_l.append(mw)
        modb_t = const.tile([128, 4], F32, tag="modb_t")
        nc.sync.dma_start(
            out=modb_t,
            in_=bass.AP(tensor=mod_b.tensor, offset=mod_b.offset,
                        ap=[[1, 128], [128, 4]]),
        )


        # ---------------- fmap staging (dense bf16, gpsimd caster) --------
        fst_l = [fst_p.tile([128, H, W], BF16, tag="fst", name=f"fst{c}")
                 for c in range(IC)]
        nc.gpsimd.dma_start(out=fst_l[0], in_=fmap[0:128, :, :])
        ident = const.tile([128, 128], BF16, tag="ident")
        make_identity(nc, ident)

        vt_l = [const.tile([128, 4, TY, PXV], BF16, tag=f"vt{c}",
                           name=f"vt{c}") for c in range(IC)]
        for c in range(IC):
            vt = vt_l[c][:, :, :, :]
            for off, n in ((0, 2), (W + 2, 2)):
                nc.vector.memset(
                    bass.AP(tensor=vt.tensor, offset=vt.offset + off,
                            ap=[vt.ap[0], [PXV, 4 * TY], [1, n]]), 0.0)

        def emit_v_transform(c):
            # V rows from dense fst (img rows r = 2ty + delta - 1):
            #   V0[ty] = d(2ty-1) - d(2ty+1)   main ty=1..31, V0[0] = -d(1)
            #   V1[ty] = d(2ty)   + d(2ty+1)   all ty
            #   V2[ty] = d(2ty+1) - d(2ty)     all ty
            #   V3[ty] = d(2ty)   - d(2ty+2)   main ty=0..30, V3[31] = d(62)
            fs = fst_l[c][:, :, :]
            vt = vt_l[c][:, :, :, :]

            def rows(r0, n):
                return bass.AP(tensor=fs.tensor, offset=fs.offset + r0 * W,
                               ap=[fs.ap[0], [2 * W, n], [1, W]])

            def vout(u, ty0, n):
                return bass.AP(tensor=vt.tensor,
                               offset=vt.offset + u * TY * PXV + ty0 * PXV + 2,
                               ap=[vt.ap[0], [PXV, n], [1, W]])

            TT = nc.vector.tensor_tensor
            sub, add = mybir.AluOpType.subtract, mybir.AluOpType.add
            TT(out=vout(0, 1, 31), in0=rows(1, 31), in1=rows(3, 31), op=sub)
            nc.vector.tensor_scalar_mul(out=vout(0, 0, 1), in0=rows(1, 1),
                                        scalar1=-1.0)
            TT(out=vout(1, 0, 32), in0=rows(0, 32), in1=rows(1, 32), op=add)
            TT(out=vout(2, 0, 32), in0=rows(1, 32), in1=rows(0, 32), op=sub)
            TT(out=vout(3, 0, 31), in0=rows(0, 31), in1=rows(2, 31), op=sub)
            nc.vector.tensor_copy(out=vout(3, 31, 1), in_=rows(62, 1))

        # -------- sel = softmax(embed @ adapt_w.T + adapt_b), computed
        # redundantly on every partition (adapt_w broadcast by the DMA) so
        # no cross-partition broadcast is ever needed -------------------
        lgt = const.tile([128, NB], F32, tag="lgt")
        for n in range(NB):
            junk4 = small.tile([128, 512], F32, tag="junk", bufs=1,
                               name=f"junk4_{n}")
            nc.vector.scalar_tensor_tensor(
                out=junk4, in0=aw_b[:, n, :], scalar=1.0, in1=embed_b,
                op0=mybir.AluOpType.bypass, op1=mybir.AluOpType.mult,
                accum_out=lgt[:, n:n + 1],
            )
        nc.vector.tensor_tensor(out=lgt, in0=lgt, in1=ab_b,
                                op=mybir.AluOpType.add)
        ex = const.tile([128, NB], F32, tag="ex")
        nc.scalar.activation(out=ex, in_=lgt,
                             func=mybir.ActivationFunctionType.Exp,
                             bias=0.0, scale=1.0)
        sm = const.tile([128, 1], F32, tag="sm")
        nc.vector.tensor_reduce(out=sm, in_=ex, axis=mybir.AxisListType.X,
                                op=mybir.AluOpType.add)
        rs = const.tile([128, 1], F32, tag="rs")
        nc.vector.reciprocal(out=rs, in_=sm)
        sel_b = const.tile([128, NB], F32, tag="sel_b")
        nc.vector.tensor_scalar_mul(out=sel_b, in0=ex, scalar1=rs)

        # mod/mscale tiles (DMAs + compute emitted after the first weight
        # block so the sync ring streams wbig-c0 first and the DVE runs the
        # sel chain unobstructed)
        mod_t = const.tile([128, 4], F32, tag="mod_t")
        msc = const.tile([128, 4], F32, tag="msc")
        msq_t = const.tile([128, 4], F32, tag="msq_t")
        msq_f = [const.tile([128, 128], F32, tag=f"m9f{c}", name=f"m9f{c}")
                 for c in range(IC)]
        msq9 = [None] * IC

        def emit_mod_chain():
            for c in range(4):
                junk = small.tile([128, 512], F32, tag="junk", bufs=1,
                                  name=f"junkm{c}")
                nc.vector.scalar_tensor_tensor(
                    out=junk, in0=mw_l[c], scalar=1.0, in1=embed_b,
                    op0=mybir.AluOpType.bypass, op1=mybir.AluOpType.mult,
                    accum_out=mod_t[:, c:c + 1],
                )
            nc.vector.scalar_tensor_tensor(
                out=msc, in0=mod_t, scalar=1.0, in1=modb_t,
                op0=mybir.AluOpType.add, op1=mybir.AluOpType.add,
            )
            nc.vector.tensor_tensor(out=msq_t, in0=msc, in1=msc,
                                    op=mybir.AluOpType.mult)
            nc.gpsimd.dma_start(
                out=bass.AP(tensor=msq_dram.tensor, offset=msq_dram.offset,
                            ap=[[1, 128], [128, 4]]),
                in_=msq_t,
            )
            for c in range(IC):
                nc.gpsimd.dma_start(
                    out=msq_f[c],
                    in_=bass.AP(tensor=msq_dram.tensor,
                                offset=msq_dram.offset + c * 128,
                                ap=[[0, 128], [1, 128]]),
                )

        inv_norm = const.tile([128, OC], F32, tag="inv_norm")
        inv_half = const.tile([128, OC], F32, tag="inv_half")

        wct_d, uw_d, ntp_d = {}, {}, {}

        def weights_block(q, c, emit_norm=True):
            if c == 0:
                wct_d[q] = [wct_p.tile([128, KS * KS, 128], BF16,
                                       tag=f"wct{cc}", name=f"wct{cc}_{q}")
                            for cc in range(IC)]
                uw_d[q] = [uw_p.tile([128, 6, 128], BF16, tag=f"uw{cc}",
                                     name=f"uw{cc}_{q}") for cc in range(IC)]
                ntp_d[q] = small.tile([128, IC], F32, tag="ntp",
                                      name=f"ntp{q}")
            wct_q, uw_q, ntp = wct_d[q], uw_d[q], ntp_d[q]
            wbig = wbank_p.tile([128, NB, 128 * KS * KS], F32, tag="wbig",
                                name=f"wbig{q}_{c}")
            nsplit = NB if q == 0 else 1
            bper = NB // nsplit
            for sp in range(nsplit):
                nc.sync.dma_start(
                    out=wbig[:, sp * bper:(sp + 1) * bper, :],
                    in_=bass.AP(
                        tensor=weights.tensor,
                        offset=(weights.offset + q * 128 * I * KS * KS
                                + sp * bper * O * I * KS * KS
                                + c * 128 * KS * KS),
                        ap=[[I * KS * KS, 128], [O * I * KS * KS, bper],
                            [1, 128 * KS * KS]],
                    ),
                )
            # fold sel into the bf16 cast (ACT, per-partition fp32 scale)
            wsc = wsc_p.tile([128, NB, 128 * KS * KS], BF16, tag="wsc",
                             name=f"wsc{q}_{c}")
            for n in range(NB):
                if q == 0 and n >= 2:
                    nc.vector.tensor_scalar_mul(out=wsc[:, n, :],
                                                in0=wbig[:, n, :],
                                                scalar1=sel_b[:, n:n + 1])
                else:
                    nc.scalar.activation(
                        out=wsc[:, n, :], in_=wbig[:, n, :],
                        func=mybir.ActivationFunctionType.Copy,
                        scale=sel_b[:, n:n + 1],
                    )
            # combine: pure bf16 TT adds (fast path); v dense [o, i, tap]
            t0 = small.tile([128, 128 * KS * KS], BF16, tag="t0", bufs=1)
            nc.vector.tensor_tensor(out=t0, in0=wsc[:, 0, :],
                                    in1=wsc[:, 1, :], op=mybir.AluOpType.add)
            t1 = small.tile([128, 128 * KS * KS], BF16, tag="t1", bufs=1)
            nc.vector.tensor_tensor(out=t1, in0=wsc[:, 2, :],
                                    in1=wsc[:, 3, :], op=mybir.AluOpType.add)
            v = v_p.tile([128, 128, KS * KS], BF16, tag="v",
                         name=f"v{q}_{c}")
            nc.vector.tensor_tensor(
                out=v, in0=t0.rearrange("p (i j) -> p i j", j=KS * KS),
                in1=t1.rearrange("p (i j) -> p i j", j=KS * KS),
                op=mybir.AluOpType.add)
            # taps [o,i]->[i,o], 3 groups of 3 into just-freed PSUM slots
            va = v[:, :, :]
            pts = []
            for g in range(3):
                pt = ps_p.tile([128, 3, 128], BF16, tag=f"m{g}_{c % 2}",
                               name=f"pt{g}_{q}_{c}")
                for j in range(3 * g, 3 * g + 3):
                    nc.tensor.transpose(
                        pt[:, j - 3 * g, :],
                        bass.AP(tensor=va.tensor, offset=va.offset + j,
                                ap=[va.ap[0], [KS * KS, 128]]),
                        ident)
                pts.append(pt)

            def tail():
                # ACT evacuation applies mscale (i on partitions)
                for g, pt in enumerate(pts):
                    nc.scalar.activation(
                        out=wct_q[c][:, 3 * g:3 * g + 3, :], in_=pt,
                        func=mybir.ActivationFunctionType.Copy,
                        scale=msc[:, c:c + 1],
                    )
                if q == 0:
                    emit_v_transform(c)
                # winograd weight transform (pure TT adds)
                st = small.tile([128, KS, 128], BF16, tag="st", bufs=1)
                nc.vector.tensor_tensor(out=st, in0=wct_q[c][:, 0:3, :],
                                        in1=wct_q[c][:, 6:9, :],
                                        op=mybir.AluOpType.add)
                nc.vector.tensor_tensor(out=uw_q[c][:, 0:3, :], in0=st,
                                        in1=wct_q[c][:, 3:6, :],
                                        op=mybir.AluOpType.add)
                nc.vector.tensor_tensor(out=uw_q[c][:, 3:6, :], in0=st,
                                        in1=wct_q[c][:, 3:6, :],
                                        op=mybir.AluOpType.subtract)
                weights_norm(q, c, v)

            if emit_norm:
                tail()
            return wbig, tail

        def weights_norm(q, c, v):
            # demod norm partial (off the critical path): sum v^2 * msq[i]
            ntp = ntp_d[q]
            if q == 0:
                msq9[c] = const.tile([128, 128], BF16, tag=f"m9b{c}",
                                     name=f"m9b{c}")
                nc.vector.tensor_copy(out=msq9[c], in_=msq_f[c])
            m9ap = msq9[c][:, :]
            m9b = bass.AP(tensor=m9ap.tensor, offset=m9ap.offset,
                          ap=[m9ap.ap[0], [1, 128], [0, KS * KS]])
            vm = vm_p.tile([128, 128, KS * KS], BF16, tag="vm")
            nc.vector.tensor_tensor(out=vm, in0=v, in1=m9b,
                                    op=mybir.AluOpType.mult)
            nc.vector.scalar_tensor_tensor(
                out=vm, in0=vm, scalar=1.0, in1=v,
                op0=mybir.AluOpType.bypass, op1=mybir.AluOpType.mult,
                accum_out=ntp[:, c:c + 1],
            )

        emit_mod_chain()
        wbig00, _ = weights_block(0, 0)
        # hold fst c1-3 DMAs until the first weight chunk has landed, so
        # the critical-path prologue traffic is not starved for bandwidth
        dummy = const.tile([1, 4], F32, tag="dummy")
        nc.gpsimd.tensor_copy(out=dummy[:, 0:1], in_=wbig00[0:1, NB - 1, 0:1])
        nc.gpsimd.dma_start(out=fst_l[1], in_=fmap[128:256, :, :])
        for c in range(1, IC):
            wbig0c, _ = weights_block(0, c)
            if c + 1 < IC:
                nc.gpsimd.tensor_copy(out=dummy[:, c:c + 1],
                                      in_=wbig0c[0:1, NB - 1, 0:1])
                nc.gpsimd.dma_start(
                    out=fst_l[c + 1],
                    in_=fmap[(c + 1) * 128:(c + 2) * 128, :, :])

        for q in range(OC):
            wct_q, uw_q, ntp = wct_d[q], uw_d[q], ntp_d[q]
            # inv_norm[q] = rsqrt(clip(sum_c ntp, EPS)); inv_half = half
            nt1 = small.tile([128, 1], F32, tag="nt1")
            nc.vector.tensor_reduce(out=nt1, in_=ntp, axis=mybir.AxisListType.X,
                                    op=mybir.AluOpType.add)
            nc.vector.tensor_scalar_max(out=nt1, in0=nt1, scalar1=EPS)
            nc.scalar.sqrt(out=nt1, in_=nt1)
            nc.vector.reciprocal(out=inv_norm[:, q:q + 1], in_=nt1)
            nc.vector.tensor_scalar_mul(out=inv_half[:, q:q + 1],
                                        in0=inv_norm[:, q:q + 1], scalar1=0.5)

            def tap(c, u, kx):
                if u == 0:
                    return wct_q[c][:, kx, :]
                if u == 3:
                    return wct_q[c][:, 6 + kx, :]
                return uw_q[c][:, (u - 1) * 3 + kx, :]

            def gemm_mm(mt, tb, c, u, kx):
                nc.tensor.matmul(
                    mt[u],
                    tap(c, u, kx),
                    vt_l[c][:, u, tb * TB:(tb + 1) * TB, kx + 1:kx + 1 + W],
                    start=(c == 0 and kx == 0),
                    stop=(c == IC - 1 and kx == KS - 1),
                    skip_group_check=True,
                )

            def out_block(tb, mt):
                # output transform: z = s_u * inv_norm * M (ACT), pure TTs
                z = z_p.tile([128, 4, 512], BF16, tag="z")
                for u in range(4):
                    sc = inv_half if u in (1, 2) else inv_norm
                    nc.scalar.activation(
                        out=z[:, u, :], in_=mt[u],
                        func=mybir.ActivationFunctionType.Copy,
                        scale=sc[:, q:q + 1],
                    )
                s = z_p.tile([128, 512], BF16, tag="s", bufs=1)
                d = z_p.tile([128, 512], BF16, tag="d", bufs=1)
                nc.vector.tensor_tensor(out=s, in0=z[:, 1, :], in1=z[:, 2, :],
                                        op=mybir.AluOpType.add)
                nc.vector.tensor_tensor(out=d, in0=z[:, 1, :], in1=z[:, 2, :],
                                        op=mybir.AluOpType.subtract)
                # DVE writes the row interleave directly (strided fp32
                # stays in 1x mode); one contiguous fp32 DMA per block
                ob = ob_p.tile([128, 2 * TB, W], F32, tag="ob")
                oba = ob[:, :, :]
                y_even = bass.AP(tensor=oba.tensor, offset=oba.offset,
                                 ap=[oba.ap[0], [2 * W, TB], [1, W]])
                y_odd = bass.AP(tensor=oba.tensor, offset=oba.offset + W,
                                ap=[oba.ap[0], [2 * W, TB], [1, W]])
                nc.vector.tensor_tensor(out=y_even, in0=s, in1=z[:, 0, :],
                                        op=mybir.AluOpType.add)
                nc.vector.tensor_tensor(out=y_odd, in0=d, in1=z[:, 3, :],
                                        op=mybir.AluOpType.subtract)
                nc.scalar.dma_start(
                    out=out[q * 128:(q + 1) * 128,
                            tb * 2 * TB:(tb + 1) * 2 * TB, :],
                    in_=ob,
                )

            def mk_mt(tb):
                return [ps_p.tile([128, 512], F32, tag=f"m{u}_{tb % 2}",
                                  name=f"m{u}_{q}_{tb}") for u in range(4)]

            # -------- GEMM sweep: 4 u-banks per ty-block, c/kx contracted.
            # q0 runs c-OUTER over tb-pairs so matmuls start as soon as the
            # first weight chunk is ready instead of waiting for all four.
            if q == 0:
                for pair in range(NTB // 2):
                    tbs = (2 * pair, 2 * pair + 1)
                    mts = {tb: mk_mt(tb) for tb in tbs}
                    for c in range(IC):
                        for tb in tbs:
                            for u in range(4):
                                for kx in range(KS):
                                    gemm_mm(mts[tb], tb, c, u, kx)
                    for tb in tbs:
                        out_block(tb, mts[tb])
                        weights_block(1, tb)
            else:
                for tb in range(NTB):
                    mt = mk_mt(tb)
                    for c in range(IC):
                        for u in range(4):
                            for kx in range(KS):
                                gemm_mm(mt, tb, c, u, kx)
                    out_block(tb, mt)
                    if q + 1 < OC:
                        weights_block(q + 1, tb)


def _get_nc():
    if "nc" not in _CACHED:
        _CACHED["nc"] = _build()
    return _CACHED["nc"]


def _run(inputs, trace=False):
    nc = _get_nc()
    fmap = np.ascontiguousarray(inputs["fmap"], dtype=np.float32)
    embed = np.ascontiguousarray(inputs["embed"], dtype=np.float32)
    shared = {
        "weights": np.ascontiguousarray(inputs["weights"], dtype=np.float32),
        "mod_w": np.ascontiguousarray(inputs["mod_w"], dtype=np.float32),
        "mod_b": np.ascontiguousarray(inputs["mod_b"], dtype=np.float32),
        "adapt_w": np.ascontiguousarray(inputs["adapt_w"], dtype=np.float32),
        "adapt_b": np.ascontiguousarray(inputs["adapt_b"], dtype=np.float32),
    }
    b = fmap.shape[0]
    in_maps = [
        {"fmap": np.ascontiguousarray(fmap[c]),
         "embed": np.ascontiguousarray(embed[c]), **shared}
        for c in range(b)
    ]
    res = run_bass_kernel_spmd(nc, in_maps, core_ids=list(range(b)),
                               trace=trace)
    _CACHED["last_res"] = res
    outs = np.stack([res.results[c]["out"] for c in range(b)], axis=0)
    return outs.astype(np.float32), res.exec_time_ns


def kernel(**inputs):
    out, _ = _run(inputs, trace=False)
    return out


def kernel_traced(**inputs):
    return _run(inputs, trace=True)
